# revision 31
# baseline (speedup 1.0000x reference)
"""Windowed-causal self-attention (RoPE + QK-RMSNorm + value-embedding gate)
for Trainium2, distributed over 8 NeuronCores.

Sharding: core = (batch b, head-group hg); b = core//4, hg = core%4.
Each core owns 4 of the 16 heads (a 512-wide slice of the QKV output
channels / Wproj input rows) for one batch element.  Wproj is
row-parallel, so each core emits a full-width (T, C) partial product and
the host sums the 4 partials per batch (the gather step).

All matmul operands are fp16 (10-bit mantissa, full PE rate, fp32 PSUM
accumulation); vector/scalar math is fp32.  Softmax runs max-free: row
maxima are bounded (|S| <= 1.44*sqrt(128) ~ 16.3), so exp(S - 9) never
overflows fp16 and the shift cancels in the normalization.
"""

import math
import os
import sys

import numpy as np

# The device kernel runs through jax/PJRT on the axon-tunneled NeuronCores;
# a JAX_PLATFORMS=cpu pin would hide them.
if "cpu" in os.environ.get("JAX_PLATFORMS", "") and "jax" not in sys.modules:
    os.environ.pop("JAX_PLATFORMS", None)

for _p in ("/opt/trn_rl_repo", "/root/.axon_site/_ro/trn_rl_repo"):
    if _p not in sys.path:
        sys.path.append(_p)

import concourse.bass as bass  # noqa: E402,F401
import concourse.mybir as mybir  # noqa: E402
import concourse.tile as tile  # noqa: E402
from concourse import bacc  # noqa: E402
from concourse.bass_utils import run_bass_kernel_spmd  # noqa: E402

B, T, C = 2, 2048, 2048
H, D = 16, 128
NCORES = 8
HG = 4            # head groups (cores per batch)
HPG = H // HG     # heads per core = 4
M = HPG * D       # per-core channel slice = 512
WIN = 1024        # window_left
F16, F32 = mybir.dt.float16, mybir.dt.float32
F32R = mybir.dt.float32r
AF = mybir.ActivationFunctionType
OP = mybir.AluOpType

P = 128
TC = 512          # token chunk (projection matmul moving dim)
NTC = T // TC     # 4
TS = 128          # token sub-chunk (v rows / output rows)
NTS = T // TS     # 16
CCH = C // P      # 16 contraction chunks
IC = 512          # attention query chunk
NIC = T // IC     # 4
SCC = 1.44 / math.sqrt(D)   # 1.2*1.2 qk scale folded with 1/sqrt(D)
EXP_SHIFT = -9.0

_PROGRAM = None
LAST_RESULTS = None

# All ACT functions this kernel uses (Exp, Ln, Square, Copy) live together in
# the "natural_log_exp_and_others" table set, but bacc's table-selection pass
# assigns each activation the first set containing its function, which makes
# consecutive Ln/Exp ops ping-pong table loads (1.3us each).  Restrict every
# other set's membership so the shared set is the unique choice and the load
# is hoisted to a single instruction.
_ACT_SET = "natural_log_exp_and_others"
_ORIG_GAT = bacc.get_activation_tables


def _patched_gat(arch):
    tabs = _ORIG_GAT(arch)
    funcs = {AF.Exp, AF.Ln, AF.Square, AF.Copy, AF.Identity}
    assert funcs <= tabs[_ACT_SET], (funcs, tabs[_ACT_SET])
    return {
        name: (set(s) if name == _ACT_SET else set(s) - funcs)
        for name, s in tabs.items()
    }


bacc.get_activation_tables = _patched_gat


def _build_program(phases="ABC"):
    nc = bacc.Bacc("TRN2", target_bir_lowering=False, debug=False)

    xT = nc.dram_tensor("xT", [C, T], F16, kind="ExternalInput")
    wqT = nc.dram_tensor("wqT", [C, M], F16, kind="ExternalInput")
    wkT = nc.dram_tensor("wkT", [C, M], F16, kind="ExternalInput")
    wvT = nc.dram_tensor("wvT", [C, M], F16, kind="ExternalInput")
    wpT = nc.dram_tensor("wpT", [M, C], F16, kind="ExternalInput")
    wgT = nc.dram_tensor("wgT", [12, HPG], F16, kind="ExternalInput")
    ve3 = nc.dram_tensor("ve3", [T, M], F16, kind="ExternalInput")
    cosT = nc.dram_tensor("cosT", [D // 2, T], F32, kind="ExternalInput")
    sinT = nc.dram_tensor("sinT", [D // 2, T], F32, kind="ExternalInput")
    masks = nc.dram_tensor("masks", [P, 8, IC], mybir.dt.bfloat16, kind="ExternalInput")
    out = nc.dram_tensor("out", [T, C], F32, kind="ExternalOutput")

    with tile.TileContext(nc) as tc:
        with (
            tc.tile_pool(name="consts", bufs=1) as consts,
            tc.tile_pool(name="resid", bufs=1) as resid,
            tc.tile_pool(name="work", bufs=3) as work,
        ):
            ones_col = consts.tile([P, 1], F16, tag="ones_col")
            nc.vector.memset(ones_col[:], 1.0)
            ones_f32 = consts.tile([P, 1], F32, tag="ones_f32")
            nc.vector.memset(ones_f32[:], 1.0)
            ones_col_r = consts.tile([P, 1], F32R, tag="ones_col_r")
            nc.vector.tensor_copy(ones_col_r[:], ones_f32[:])
            ones_row_f32 = consts.tile([1, P], F32, tag="ones_row_f32")
            nc.vector.memset(ones_row_f32[:], 1.0)
            ones_row_r = consts.tile([1, P], F32R, tag="ones_row_r")
            nc.vector.tensor_copy(ones_row_r[:], ones_row_f32[:])
            ones_row = consts.tile([1, P], F16, tag="ones_row")
            nc.vector.memset(ones_row[:], 1.0)
            bias_sh = consts.tile([P, 1], F32, tag="bias_sh")
            nc.vector.memset(bias_sh[:], EXP_SHIFT)
            eps_sb = consts.tile([P, 1], F32, tag="eps")
            nc.vector.memset(eps_sb[:], 1e-6)

            qT = [resid.tile([P, T], F16, tag=f"qT{h}", name=f"qT{h}") for h in range(HPG)]
            kT = [resid.tile([P, T], F16, tag=f"kT{h}", name=f"kT{h}") for h in range(HPG)]
            v16 = resid.tile([P, NTS, M], F32R, tag="v16")

            # ------------ Phase A: projections + rope + rmsnorm + gate
            with (
                tc.tile_pool(name="pa", bufs=1) as pa,
                tc.tile_pool(name="xp", bufs=2) as xp,
                tc.tile_pool(name="psA", bufs=1, space="PSUM") as ps,
            ):
                # cos/sin duplicated across both halves for full-width rope
                cc2 = pa.tile([P, T], F32, tag="cc2")
                nc.sync.dma_start(cc2[:64], cosT[:])
                nc.sync.dma_start(cc2[64:], cosT[:])
                ss2 = pa.tile([P, T], F32, tag="ss2")
                nc.sync.dma_start(ss2[:64], sinT[:])
                nc.sync.dma_start(ss2[64:], sinT[:])
                wq_sb = pa.tile([P, CCH, M], F16, tag="wq")
                nc.sync.dma_start(
                    wq_sb[:], wqT[:].rearrange("(co p) m -> p co m", p=P)
                )
                wk_sb = pa.tile([P, CCH, M], F16, tag="wk")
                nc.sync.dma_start(
                    wk_sb[:], wkT[:].rearrange("(co p) m -> p co m", p=P)
                )
                wv_sb = pa.tile([P, CCH, M], F16, tag="wv")
                nc.sync.dma_start(
                    wv_sb[:], wvT[:].rearrange("(co p) m -> p co m", p=P)
                )
                wg_sb = pa.tile([12, HPG], F16, tag="wg")
                nc.sync.dma_start(wg_sb[:], wgT[:])

                qk_ctr = 0
                for tci in range(NTC):
                    xt = xp.tile([P, CCH, TC], F16, tag="xt")
                    nc.sync.dma_start(
                        xt[:],
                        xT[:, tci * TC:(tci + 1) * TC].rearrange(
                            "(co p) t -> p co t", p=P
                        ),
                    )
                    tsl = slice(tci * TC, (tci + 1) * TC)
                    for mc in range(HPG):
                        for w_sb, dstl in ((wq_sb, qT), (wk_sb, kT)):
                            pqk = ps.tile([P, TC], F32, tag=f"qk{qk_ctr % 2}")
                            qk_ctr += 1
                            for cc in range(CCH):
                                nc.tensor.matmul(
                                    pqk[:],
                                    w_sb[:, cc, mc * P:(mc + 1) * P],
                                    xt[:, cc, :],
                                    start=(cc == 0),
                                    stop=(cc == CCH - 1),
                                )
                            # rope: r[:64] = p[:64]*cos + p[64:]*sin
                            #       r[64:] = p[64:]*cos - p[:64]*sin
                            # full-width products; the sin product goes to a
                            # PSUM temp so the cross-half add/sub mixes
                            # memories (the both-SBUF same-base rule).
                            qr = work.tile([P, TC], F32, tag="qr")
                            nc.vector.tensor_mul(qr[:], pqk[:], cc2[:, tsl])
                            prope = ps.tile([P, TC], F32, tag="rope")
                            nc.vector.tensor_mul(prope[:], pqk[:], ss2[:, tsl])
                            nc.vector.tensor_add(qr[:64], qr[:64], prope[64:])
                            nc.vector.tensor_sub(qr[64:], qr[64:], prope[:64])
                            # rms-norm: scale cols by 1/sqrt(mean(qr^2)+eps)
                            sq16 = work.tile([P, TC], F16, tag="sq")
                            nc.scalar.square(sq16[:], qr[:])
                            pss = ps.tile([1, TC], F32, tag="ss")
                            nc.tensor.matmul(
                                pss[:], ones_col[:], sq16[:], start=True, stop=True
                            )
                            # rstd = exp(-0.5*ln(mean+eps)); ln/exp share
                            # one ACT table set (sqrt would force a reload)
                            s32 = work.tile([1, TC], F32, tag="s32")
                            nc.scalar.activation(
                                s32[:], pss[:], AF.Ln,
                                bias=eps_sb[:1], scale=1.0 / D,
                            )
                            r32 = work.tile([1, TC], F32, tag="r32")
                            nc.scalar.activation(r32[:], s32[:], AF.Exp, scale=-0.5)
                            r16 = work.tile([1, TC], F16, tag="r16")
                            nc.vector.tensor_copy(r16[:], r32[:])
                            prb = ps.tile([P, TC], F32, tag="rqb")
                            nc.tensor.matmul(
                                prb[:], ones_row[:], r16[:], start=True, stop=True
                            )
                            nc.vector.tensor_mul(dstl[mc][:, tsl], qr[:], prb[:])

                    for tsi in range(TC // TS):
                        ts_g = tci * (TC // TS) + tsi
                        pv = ps.tile([P, M], F32, tag=f"vv{ts_g % 2}")
                        for cc in range(CCH):
                            nc.tensor.matmul(
                                pv[:],
                                xt[:, cc, tsi * TS:(tsi + 1) * TS],
                                wv_sb[:, cc, :],
                                start=(cc == 0),
                                stop=(cc == CCH - 1),
                            )
                        pg = ps.tile([P, HPG], F32, tag="g")
                        nc.tensor.matmul(
                            pg[:],
                            xt[:12, 0, tsi * TS:(tsi + 1) * TS],
                            wg_sb[:],
                            start=True,
                            stop=True,
                        )
                        # sigmoid = 1/(1+exp(-z)) keeps ACT on the exp table set
                        ge = work.tile([P, HPG], F32, tag="ge")
                        nc.scalar.activation(ge[:], pg[:], AF.Exp, scale=-1.0)
                        gd = work.tile([P, HPG], F32, tag="gd")
                        nc.vector.tensor_scalar_add(gd[:], ge[:], 1.0)
                        gate = work.tile([P, HPG], F32, tag="gate")
                        nc.vector.reciprocal(gate[:], gd[:])
                        vet = work.tile([P, M], F16, tag="ve")
                        nc.sync.dma_start(
                            vet[:], ve3[ts_g * TS:(ts_g + 1) * TS, :]
                        )
                        for h in range(HPG):
                            hsl = slice(h * P, (h + 1) * P)
                            nc.vector.scalar_tensor_tensor(
                                v16[:, ts_g, hsl],
                                vet[:, hsl],
                                gate[:, h:h + 1],
                                pv[:, hsl],
                                OP.mult,
                                OP.add,
                            )

            # ------------ Phase B: windowed attention (ic-outer so the
            # output projection for finished i-chunks overlaps later chunks)
            if "B" not in phases:
                nc.compile()
                return nc
            with (
                tc.tile_pool(name="expp", bufs=4) as expp,
                tc.tile_pool(name="pc", bufs=1) as pc,
                tc.tile_pool(name="stg", bufs=2) as stg,
                tc.tile_pool(name="psB", bufs=1, space="PSUM") as psb,
                tc.tile_pool(name="psC", bufs=1, space="PSUM") as psc,
            ):
                yT = pc.tile([P, HPG, T], F16, tag="yT")
                mk_sb = pc.tile([P, 8, IC], mybir.dt.bfloat16, tag="masks")
                nc.sync.dma_start(mk_sb[:], masks[:])
                wp_sb = pc.tile([P, HPG, C], F16, tag="wp")
                nc.sync.dma_start(
                    wp_sb[:], wpT[:].rearrange("(mo p) n -> p mo n", p=P)
                )
                st_ctr = 0
                for ici in range(NIC):
                    i0 = ici * IC
                    isl = slice(i0, i0 + IC)
                    jts = list(range(max(0, (i0 - WIN) // P), (i0 + IC) // P))
                    # per-tile valid column range [lo, hi): causal tiles
                    # only reach columns >= d_off, window tiles only columns
                    # < w_off+128.  The d_off==0 tile is always full-width and
                    # goes first so its start=True write covers the whole
                    # psum bank before partial-range accumulation.
                    tiles = []
                    for jt in jts:
                        d_off = jt * P - i0
                        w_off = d_off + WIN
                        if d_off >= 0:
                            tiles.append((jt, d_off, IC, d_off // P))
                        elif w_off < IC:
                            tiles.append((jt, 0, w_off + P, 4 + w_off // P))
                        else:
                            tiles.append((jt, 0, IC, None))
                    tiles.sort(key=lambda t: (t[1] != 0 or t[2] != IC, ))
                    assert tiles[0][1] == 0 and tiles[0][2] == IC
                    for h in range(HPG):
                        hsl = slice(h * P, (h + 1) * P)
                        ih = ici * HPG + h
                        ppv = psb.tile([P, IC], F32, tag=f"pv{ih % 2}")
                        pden = psb.tile([1, IC], F32, tag="den")
                        for idx, (jt, lo, hi, mi) in enumerate(tiles):
                            csl = slice(lo, hi)
                            qsl = slice(i0 + lo, i0 + hi)
                            pst = psb.tile([P, IC], F32, tag=f"st{st_ctr % 3}")
                            st_ctr += 1
                            nc.tensor.matmul(
                                pst[:, csl],
                                kT[h][:, jt * P:(jt + 1) * P],
                                qT[h][:, qsl],
                                start=True,
                                stop=True,
                            )
                            if mi is not None:
                                # only the 128-wide band where the diagonal
                                # crosses the tile actually needs masking
                                b_lo = lo if mi < 4 else hi - P
                                msl = slice(b_lo, min(b_lo + P, IC))
                                nc.vector.tensor_add(
                                    pst[:, msl], pst[:, msl], mk_sb[:, mi, msl]
                                )
                            e16 = expp.tile([P, IC], F32R, tag="exp")
                            nc.scalar.activation(
                                e16[:, csl], pst[:, csl], AF.Exp,
                                bias=bias_sh[:], scale=SCC,
                            )
                            nc.tensor.matmul(
                                ppv[:, csl],
                                v16[:, jt, hsl],
                                e16[:, csl],
                                start=(idx == 0),
                                stop=(idx == len(tiles) - 1),
                            )
                            nc.tensor.matmul(
                                pden[:, csl],
                                ones_col_r[:],
                                e16[:, csl],
                                start=(idx == 0),
                                stop=(idx == len(tiles) - 1),
                            )
                        r32 = work.tile([1, IC], F32, tag="br32")
                        nc.vector.reciprocal(r32[:], pden[:])
                        r32r = work.tile([1, IC], F32R, tag="br32r")
                        nc.vector.tensor_copy(r32r[:], r32[:])
                        pdb = psb.tile([P, IC], F32, tag=f"st{st_ctr % 3}")
                        st_ctr += 1
                        nc.tensor.matmul(
                            pdb[:], ones_row_r[:], r32r[:], start=True, stop=True
                        )
                        db = work.tile([P, IC], F32, tag="dbsb")
                        nc.scalar.copy(db[:], pdb[:])
                        nc.vector.tensor_mul(yT[:, h, isl], ppv[:], db[:])


                # ---- output projection, serial after attention
                if "C" in phases:
                    for ts_g in range(NTS):
                        stage = stg.tile([P, C], F32, tag="stage")
                        for nck in range(C // 512):
                            po = psc.tile([P, 512], F32, tag=f"o{nck % 2}")
                            for mh in range(HPG):
                                nc.tensor.matmul(
                                    po[:],
                                    yT[:, mh, ts_g * TS:(ts_g + 1) * TS],
                                    wp_sb[:, mh, nck * 512:(nck + 1) * 512],
                                    start=(mh == 0),
                                    stop=(mh == HPG - 1),
                                )
                            if nck % 2 == 0:
                                nc.scalar.copy(stage[:, nck * 512:(nck + 1) * 512], po[:])
                            else:
                                nc.vector.tensor_copy(stage[:, nck * 512:(nck + 1) * 512], po[:])
                        nc.sync.dma_start(
                            out[ts_g * TS:(ts_g + 1) * TS, :], stage[:]
                        )

    nc.compile()
    return nc

def _get_program():
    global _PROGRAM
    if _PROGRAM is None:
        _PROGRAM = _build_program()
    return _PROGRAM


def _build_masks(window_left):
    jj = np.arange(P)[:, None]
    ii = np.arange(IC)[None, :]
    m = np.empty((8, P, IC), np.float32)
    for ci in range(4):      # causal tiles: valid iff jj + ci*128 <= ii
        m[ci] = np.where(jj + ci * P <= ii, 0.0, -1e9)
    for wi in range(4):      # window tiles: valid iff jj + wi*128 >= ii
        m[4 + wi] = np.where(jj + wi * P >= ii, 0.0, -1e9)
    # device layout [P, 8, IC], bf16
    import ml_dtypes
    return np.ascontiguousarray(m.transpose(1, 0, 2)).astype(ml_dtypes.bfloat16)


def kernel(**inputs):
    global LAST_RESULTS
    x = np.asarray(inputs["x"], np.float32)
    ve = np.asarray(inputs["ve"], np.float32)
    Wq = np.asarray(inputs["Wq"], np.float32)
    Wk = np.asarray(inputs["Wk"], np.float32)
    Wv = np.asarray(inputs["Wv"], np.float32)
    Wproj = np.asarray(inputs["Wproj"], np.float32)
    Wgate = np.asarray(inputs["Wgate"], np.float32)
    cos = np.asarray(inputs["cos"], np.float32)
    sin = np.asarray(inputs["sin"], np.float32)
    window_left = int(inputs["window_left"])
    assert window_left == WIN, f"kernel compiled for window_left={WIN}"
    assert x.shape == (B, T, C)

    nc = _get_program()

    wqT = np.ascontiguousarray(Wq.T).astype(np.float16)
    wkT = np.ascontiguousarray(Wk.T).astype(np.float16)
    wvT = np.ascontiguousarray(Wv.T).astype(np.float16)
    wpT = np.ascontiguousarray(Wproj.T).astype(np.float16)
    wgT = np.ascontiguousarray(Wgate[:, :12].T).astype(np.float16)  # (12, 16)
    cosT = np.ascontiguousarray(cos[:, 0, :].T)  # (64, T)
    sinT = np.ascontiguousarray(sin[:, 0, :].T)
    masks = _build_masks(window_left)
    xT = [np.ascontiguousarray(x[b].T).astype(np.float16) for b in range(B)]
    ve3 = (3.0 * ve).astype(np.float16)

    in_maps = []
    for core in range(NCORES):
        b, hg = core // HG, core % HG
        msl = slice(hg * M, (hg + 1) * M)
        in_maps.append({
            "xT": xT[b],
            "wqT": np.ascontiguousarray(wqT[:, msl]),
            "wkT": np.ascontiguousarray(wkT[:, msl]),
            "wvT": np.ascontiguousarray(wvT[:, msl]),
            "wpT": np.ascontiguousarray(wpT[msl, :]),
            "wgT": np.ascontiguousarray(wgT[:, hg * HPG:(hg + 1) * HPG]),
            "ve3": np.ascontiguousarray(ve3[b][:, msl]),
            "cosT": cosT,
            "sinT": sinT,
            "masks": masks,
        })

    res = run_bass_kernel_spmd(nc, in_maps, core_ids=list(range(NCORES)))
    LAST_RESULTS = res

    y = np.zeros((B, T, C), np.float32)
    for core in range(NCORES):
        y[core // HG] += res.results[core]["out"]
    return y
